# revision 24
# baseline (speedup 1.0000x reference)
"""2-layer GAT on 8 Trainium2 NeuronCores.

Strategy: dst-shard nodes across cores; per-edge node-feature access via
dma_gather from a bf16 node table (built on device, AllGathered); segment
softmax + aggregation via one-hot selection matmuls on TensorE.

v1: gathers grouped by (chunk-of-windows, quarter) to amortize the large
fixed per-call dma_gather cost; double-buffered chunk pipeline; leaky-relu
via exp(lrelu(x)) = max(exp(x), exp(0.2x)) on ScalarE.
"""
import numpy as np
import ml_dtypes

import concourse.bacc as bacc
import concourse.bass as bass
import concourse.mybir as mybir
import concourse.tile as tile
from concourse import bass_utils

BF = ml_dtypes.bfloat16
bf16 = mybir.dt.bfloat16
f32 = mybir.dt.float32
i16 = mybir.dt.int16

N = 100000
NCORES = 8
SHARD = N // NCORES           # 12500
WIN = 128
W = (SHARD + WIN - 1) // WIN  # 98
SHARD_PAD = W * WIN           # 12544
NQ = 4
QROWS = 2 * SHARD_PAD         # 25088 rows per gather quarter (< 32768)
TROW = 128                    # bf16 elems per table row (256B)
FIN = 512
NCLS = 40
CHW = 7                       # windows per gather chunk
NCH = (W + CHW - 1) // CHW    # 14
AS1_OFF, AD1_OFF = 72, 80
AS2_OFF, AD2_OFF = 48, 56
EPS = 1e-16
AF = mybir.ActivationFunctionType
ALU = mybir.AluOpType

_CACHE = {}


# ---------------------------------------------------------------- host prep
def _prep(edge_index):
    src = np.concatenate([np.asarray(edge_index[0], np.int64),
                          np.arange(N, dtype=np.int64)])
    dst = np.concatenate([np.asarray(edge_index[1], np.int64),
                          np.arange(N, dtype=np.int64)])
    row = (src // SHARD) * SHARD_PAD + (src % SHARD)
    quarter = row // QROWS
    core = dst // SHARD
    dstloc = dst % SHARD
    win = dstloc // WIN
    dstrel = dstloc % WIN
    chunk = win // CHW

    order = np.lexsort((dstrel, win, quarter, core))
    row_s, rel_s = row[order], dstrel[order]

    # cell = (quarter, win); static size = x128 of max count over cores
    cell_id = (core * NQ + quarter) * W + win
    counts = np.bincount(cell_id, minlength=NCORES * NQ * W) \
        .reshape(NCORES, NQ, W)
    kq = (counts.max(axis=0) + 127) // 128              # [NQ, W] blocks

    # chunk layout: for each chunk: quarters in order, cells of its windows
    cell_blk = np.zeros((NQ, W), np.int64)    # chunk-local block offset
    cq_blk = np.zeros((NCH, NQ), np.int64)    # chunk-local block offset of q
    ch_blocks = np.zeros(NCH, np.int64)
    for ch in range(NCH):
        w0, w1 = ch * CHW, min((ch + 1) * CHW, W)
        off = 0
        for q in range(NQ):
            cq_blk[ch, q] = off
            for w in range(w0, w1):
                cell_blk[q, w] = off
                off += kq[q, w]
        ch_blocks[ch] = off
    ch_blk_off = np.zeros(NCH, np.int64)      # global block offset of chunk
    ch_blk_off[1:] = np.cumsum(ch_blocks)[:-1]
    total_blocks = int(ch_blocks.sum())
    S = total_blocks * 128

    idx16 = np.zeros((NCORES, S), np.int16)
    relv = np.full((NCORES, S), -1.0, np.float32)
    starts = np.zeros(NCORES * NQ * W + 1, np.int64)
    np.cumsum(np.bincount(cell_id, minlength=NCORES * NQ * W), out=starts[1:])
    for c in range(NCORES):
        for q in range(NQ):
            for w in range(W):
                ch = w // CHW
                cid = (c * NQ + q) * W + w
                s0, s1 = starts[cid], starts[cid + 1]
                n = s1 - s0
                o = (ch_blk_off[ch] + cell_blk[q, w]) * 128
                idx16[c, o:o + n] = (row_s[s0:s1] - q * QROWS).astype(np.int16)
                relv[c, o:o + n] = rel_s[s0:s1].astype(np.float32)

    # per-window runs: (q, chunk-local blk0, kq) with kq>0; K = total blocks
    win_runs = []
    K = np.zeros(W, np.int64)
    for w in range(W):
        runs = []
        for q in range(NQ):
            if kq[q, w] > 0:
                runs.append((q, int(cell_blk[q, w]), int(kq[q, w])))
        win_runs.append(runs)
        K[w] = kq[:, w].sum()

    return {
        "idx16": idx16, "dstrel": relv, "win_runs": win_runs, "K": K,
        "ch_blocks": ch_blocks, "ch_blk_off": ch_blk_off, "cq_blk": cq_blk,
        "total_blocks": total_blocks, "S": S,
    }


def _build_inputs(meta, inputs):
    x = np.asarray(inputs["x"], np.float32)
    W1 = np.asarray(inputs["W1"], np.float32)
    W2 = np.asarray(inputs["W2"], np.float32)
    as1 = np.asarray(inputs["att_src1"], np.float32).reshape(8, 8)
    ad1 = np.asarray(inputs["att_dst1"], np.float32).reshape(8, 8)
    as2 = np.asarray(inputs["att_src2"], np.float32).reshape(NCLS)
    ad2 = np.asarray(inputs["att_dst2"], np.float32).reshape(NCLS)
    b1 = np.asarray(inputs["b1"], np.float32)
    b2 = np.asarray(inputs["b2"], np.float32)

    attsd = np.zeros((64, 16), np.float32)
    for h in range(8):
        attsd[h * 8:(h + 1) * 8, h] = as1[h]
        attsd[h * 8:(h + 1) * 8, 8 + h] = ad1[h]
    att2sd = np.stack([as2, ad2], axis=1)

    common = {
        "w1": W1.astype(BF),
        "w2": W2.astype(BF),
        "attsd": attsd.astype(BF),
        "att2sd": att2sd.astype(BF),
        "b1c": np.tile(b1[None, :], (128, 1)).astype(np.float32),
        "b2c": np.tile(b2[None, :], (128, 1)).astype(np.float32),
        "rconst": np.tile(np.arange(128, dtype=np.float32)[None, :],
                          (128, 1)).astype(BF),
        "ident": np.eye(128, dtype=np.float32).astype(BF),
    }
    S = int(meta["S"])
    maps = []
    for core in range(NCORES):
        idx = meta["idx16"][core]
        idx_in = np.tile(idx.reshape(S // 16, 16).T, (8, 1))
        drel_in = meta["dstrel"][core].reshape(S // 128, 128).T.astype(BF)
        m = dict(common)
        m["xT"] = np.ascontiguousarray(
            x[core * SHARD:(core + 1) * SHARD].T).astype(BF)
        m["idxs"] = np.ascontiguousarray(idx_in)
        m["drel"] = np.ascontiguousarray(drel_in)
        maps.append(m)
    return maps


# ---------------------------------------------------------------- bass build
def _edge_phase(nc, tc, meta, tbl_full, idxs, drel_s, rconst_s, ident_s,
                adw, layer, finalize):
    K = meta["K"]
    ch_blocks, ch_blk_off = meta["ch_blocks"], meta["ch_blk_off"]
    cq_blk, win_runs = meta["cq_blk"], meta["win_runs"]
    H = 8 if layer == 1 else 1
    VPW = 72 if layer == 1 else 41
    GW = 9 if layer == 1 else 41
    AOFF = AS1_OFF if layer == 1 else AS2_OFF
    CBMAX = int(ch_blocks.max())
    KWMAX = int(K.max())
    with tc.tile_pool(name=f"ep{layer}", bufs=2) as pool, \
         tc.tile_pool(name=f"eg{layer}", bufs=2) as gpool, \
         tc.tile_pool(name=f"ea{layer}", bufs=2) as apool, \
         tc.tile_pool(name=f"epa{layer}", bufs=2, space="PSUM") as psa, \
         tc.tile_pool(name=f"epq{layer}", bufs=2, space="PSUM") as psq, \
         tc.tile_pool(name=f"epd{layer}", bufs=2, space="PSUM") as psd:
        for ch in range(NCH):
            chb = int(ch_blocks[ch])
            gblk0 = int(ch_blk_off[ch])
            idx_t = gpool.tile([128, CBMAX * 8], i16, tag="idx")
            nc.sync.dma_start(idx_t[:, 0:chb * 8],
                              idxs[:, gblk0 * 8:(gblk0 + chb) * 8])
            g = gpool.tile([128, CBMAX, TROW], bf16, tag="gather")
            for q in range(NQ):
                b0 = int(cq_blk[ch, q])
                b1 = int(cq_blk[ch, q + 1]) if q + 1 < NQ else chb
                nbq = b1 - b0
                if nbq == 0:
                    continue
                nc.gpsimd.dma_gather(
                    out_ap=g[:, b0:b1, :],
                    in_ap=tbl_full[QROWS * q:min(QROWS * (q + 1),
                                                 NCORES * SHARD_PAD), :],
                    idxs_ap=idx_t[:, b0 * 8:b1 * 8],
                    num_idxs=nbq * 128,
                    num_idxs_reg=nbq * 128,
                    elem_size=TROW,
                    single_packet=False,
                    queue_num=q)
            w0ch = ch * CHW
            wnch = min(CHW, W - w0ch)
            accc = apool.tile([128, CHW * VPW], f32, tag="acc")
            for w in range(w0ch, w0ch + wnch):
                kw = int(K[w])
                runs = win_runs[w]
                adww = adw[:].rearrange("p (w h) -> p w h", w=W)[:, w, 0:H] \
                    if layer == 1 else adw[:, w:w + 1]
                sel = pool.tile([128, KWMAX * 128], bf16, tag="sel")
                sp = 0
                for (q, lb0, kqw) in runs:
                    nc.vector.tensor_tensor(
                        out=sel[:, sp * 128:(sp + kqw) * 128]
                            .rearrange("p (k j) -> p k j", k=kqw),
                        in0=drel_s[:, gblk0 + lb0:gblk0 + lb0 + kqw]
                            .rearrange("p (k x) -> p k x", x=1)
                            .to_broadcast([128, kqw, 128]),
                        in1=rconst_s[:].rearrange("p (x j) -> p x j", x=1)
                            .to_broadcast([128, kqw, 128]),
                        op=ALU.is_equal)
                    sp += kqw
                ep = pool.tile([128, KWMAX * H], f32, tag="ep")
                sp = 0
                for (q, lb0, kqw) in runs:
                    for k0 in range(0, kqw, 4):
                        kn = min(4, kqw - k0)
                        pst = psq.tile([128, 4 * 128], bf16, tag="selT")
                        for k in range(kn):
                            nc.tensor.transpose(
                                pst[:, k * 128:(k + 1) * 128],
                                sel[:, (sp + k0 + k) * 128:
                                    (sp + k0 + k + 1) * 128], ident_s[:])
                        selt = pool.tile([128, 4 * 128], bf16, tag="selt")
                        nc.scalar.activation(selt[:, 0:kn * 128],
                                             pst[:, 0:kn * 128], AF.Copy)
                        pad_ = psd.tile([128, 4 * H], f32, tag="adst")
                        for k in range(kn):
                            nc.tensor.matmul(
                                pad_[:, k * H:(k + 1) * H],
                                lhsT=selt[:, k * 128:(k + 1) * 128],
                                rhs=adww, start=True, stop=True)
                        nc.vector.tensor_tensor(
                            out=ep[:, (sp + k0) * H:(sp + k0 + kn) * H]
                                .rearrange("p (k h) -> p k h", k=kn),
                            in0=g[:, lb0 + k0:lb0 + k0 + kn, AOFF:AOFF + H],
                            in1=pad_[:, 0:kn * H]
                                .rearrange("p (k h) -> p k h", k=kn),
                            op=ALU.add)
                    sp += kqw
                # exp(lrelu(x)) = max(exp(x), exp(0.2x))
                pex = pool.tile([128, KWMAX * H], bf16, tag="pex")
                pex2 = pool.tile([128, KWMAX * H], bf16, tag="pex2")
                nc.scalar.activation(pex[:, 0:kw * H], ep[:, 0:kw * H], AF.Exp)
                nc.scalar.activation(pex2[:, 0:kw * H], ep[:, 0:kw * H],
                                     AF.Exp, scale=0.2)
                nc.vector.tensor_tensor(out=pex[:, 0:kw * H],
                                        in0=pex[:, 0:kw * H],
                                        in1=pex2[:, 0:kw * H], op=ALU.max)
                vp = pool.tile([128, KWMAX, VPW], bf16, tag="vp")
                sp = 0
                for (q, lb0, kqw) in runs:
                    nc.vector.tensor_tensor(
                        out=vp[:, sp:sp + kqw, :]
                            .rearrange("p k (h x) -> p k h x", h=H),
                        in0=g[:, lb0:lb0 + kqw, 0:VPW]
                            .rearrange("p k (h x) -> p k h x", h=H),
                        in1=pex[:, sp * H:(sp + kqw) * H]
                            .rearrange("p (k h x) -> p k h x", k=kqw, x=1)
                            .to_broadcast([128, kqw, H, GW]),
                        op=ALU.mult)
                    sp += kqw
                pagg = psa.tile([128, VPW], f32, tag="agg")
                for k in range(kw):
                    nc.tensor.matmul(pagg[:], lhsT=sel[:, k * 128:(k + 1) * 128],
                                     rhs=vp[:, k, :], start=(k == 0),
                                     stop=(k == kw - 1))
                nc.vector.tensor_copy(
                    accc[:, (w - w0ch) * VPW:(w - w0ch + 1) * VPW], pagg[:])
            finalize(ch, w0ch, wnch, accc)


def _build(meta):
    S = int(meta["S"])
    KT = FIN // 128

    nc = bacc.Bacc("TRN2", target_bir_lowering=False, debug=False,
                   num_devices=NCORES, num_swdge_queues=4)
    xT = nc.dram_tensor("xT", [FIN, SHARD], bf16, kind="ExternalInput")
    w1 = nc.dram_tensor("w1", [FIN, 64], bf16, kind="ExternalInput")
    w2 = nc.dram_tensor("w2", [64, NCLS], bf16, kind="ExternalInput")
    attsd = nc.dram_tensor("attsd", [64, 16], bf16, kind="ExternalInput")
    att2sd = nc.dram_tensor("att2sd", [NCLS, 2], bf16, kind="ExternalInput")
    b1c = nc.dram_tensor("b1c", [128, 64], f32, kind="ExternalInput")
    b2c = nc.dram_tensor("b2c", [128, NCLS], f32, kind="ExternalInput")
    rconst = nc.dram_tensor("rconst", [128, 128], bf16, kind="ExternalInput")
    ident = nc.dram_tensor("ident", [128, 128], bf16, kind="ExternalInput")
    idxs = nc.dram_tensor("idxs", [128, S // 16], i16, kind="ExternalInput")
    drel = nc.dram_tensor("drel", [128, S // 128], bf16, kind="ExternalInput")
    out = nc.dram_tensor("out", [SHARD_PAD, NCLS], f32, kind="ExternalOutput")

    with tile.TileContext(nc) as tc:
        with tc.tile_pool(name="dram", bufs=1, space="DRAM") as dpool, \
             tc.tile_pool(name="persist", bufs=1) as pp:
            tbl_shard = dpool.tile([SHARD_PAD, TROW], bf16)
            tbl_full, _tbl_free = tc.tile([NCORES * SHARD_PAD, TROW], bf16,
                                          space="DRAM", addr_space="Shared",
                                          name="tblfull")

            ident_s = pp.tile([128, 128], bf16)
            nc.sync.dma_start(ident_s[:], ident[:])
            rconst_s = pp.tile([128, 128], bf16)
            nc.sync.dma_start(rconst_s[:], rconst[:])
            drel_s = pp.tile([128, S // 128], bf16)
            nc.sync.dma_start(drel_s[:], drel[:])
            b1_s = pp.tile([128, 64], f32)
            nc.sync.dma_start(b1_s[:], b1c[:])
            b2_s = pp.tile([128, NCLS], f32)
            nc.sync.dma_start(b2_s[:], b2c[:])
            w2_s = pp.tile([64, NCLS], bf16)
            nc.sync.dma_start(w2_s[:], w2[:])
            att2_s = pp.tile([NCLS, 2], bf16)
            nc.sync.dma_start(att2_s[:], att2sd[:])
            adw = pp.tile([128, W * 8], bf16, tag="adw")

            # phase A: hT = W1^T x^T, aT = att^T hT; build node table layer 1
            with tc.tile_pool(name="pa", bufs=2) as pool, \
                 tc.tile_pool(name="pac", bufs=1) as cpool, \
                 tc.tile_pool(name="pap", bufs=1, space="PSUM") as psum:
                w1_s = cpool.tile([128, KT, 64], bf16)
                nc.sync.dma_start(w1_s[:], w1[:].rearrange("(k p) m -> p k m", p=128))
                att_s = cpool.tile([64, 16], bf16)
                nc.sync.dma_start(att_s[:], attsd[:])
                hT = cpool.tile([64, SHARD], bf16)
                aT = cpool.tile([16, SHARD], bf16)
                CH = 500
                for ci in range(SHARD // CH):
                    s0, s1 = ci * CH, (ci + 1) * CH
                    xt_t = pool.tile([128, KT, CH], bf16, tag="xt")
                    nc.sync.dma_start(
                        xt_t[:], xT[:, s0:s1].rearrange("(k p) n -> p k n", p=128))
                    ps = psum.tile([64, CH], f32, tag="hps")
                    for k in range(KT):
                        nc.tensor.matmul(ps[:], lhsT=w1_s[:, k, :],
                                         rhs=xt_t[:, k, :],
                                         start=(k == 0), stop=(k == KT - 1))
                    nc.vector.tensor_copy(hT[:, s0:s1], ps[:])
                    ps2 = psum.tile([16, CH], f32, tag="aps")
                    nc.tensor.matmul(ps2[:], lhsT=att_s[:], rhs=hT[:, s0:s1],
                                     start=True, stop=True)
                    nc.vector.tensor_copy(aT[:, s0:s1], ps2[:])
                WFULL = (SHARD // 128) & ~3  # full 4-window groups
                for w0 in range(0, WFULL, 4):
                    stg = pool.tile([128, 4, TROW], bf16, tag="stg")
                    pt = psum.tile([128, 4 * 64], bf16, tag="tp")
                    pt2 = psum.tile([128, 4 * 16], bf16, tag="tp2")
                    for i in range(4):
                        n0 = (w0 + i) * 128
                        nc.tensor.transpose(pt[:, i * 64:i * 64 + 64],
                                            hT[:, n0:n0 + 128],
                                            ident_s[0:64, 0:64])
                        nc.tensor.transpose(pt2[:, i * 16:i * 16 + 16],
                                            aT[:, n0:n0 + 128],
                                            ident_s[0:16, 0:16])
                    nc.vector.tensor_copy(
                        stg[:, :, 0:72].rearrange("p w (h c) -> p w h c", h=8)
                        [:, :, :, 0:8],
                        pt[:].rearrange("p (w h c) -> p w h c", w=4, h=8))
                    nc.vector.memset(
                        stg[:, :, 0:72].rearrange("p w (h c) -> p w h c", h=8)
                        [:, :, :, 8], 1.0)
                    nc.scalar.activation(
                        stg[:, :, AS1_OFF:AS1_OFF + 16],
                        pt2[:].rearrange("p (w c) -> p w c", w=4),
                        AF.Copy)
                    nc.vector.memset(stg[:, :, AS1_OFF + 16:TROW], 0.0)
                    nc.sync.dma_start(
                        tbl_shard[:].rearrange("(w p) t -> p w t", p=128)
                        [:, w0:w0 + 4, :],
                        stg[:])
                for w in range(WFULL, W):
                    stg1 = pool.tile([128, TROW], bf16, tag="stg1")
                    n0 = w * 128
                    nw = min(128, SHARD - n0)
                    if nw < 128:
                        nc.vector.memset(stg1[:], 0.0)
                    pt = psum.tile([128, 64], bf16, tag="tp1")
                    nc.tensor.transpose(pt[0:nw, 0:64], hT[:, n0:n0 + nw],
                                        ident_s[0:64, 0:64])
                    nc.vector.tensor_copy(
                        stg1[0:nw, 0:72].rearrange("p (h c) -> p h c", h=8)[:, :, 0:8],
                        pt[0:nw, 0:64].rearrange("p (h c) -> p h c", h=8))
                    nc.vector.memset(
                        stg1[:, 0:72].rearrange("p (h c) -> p h c", h=8)[:, :, 8], 1.0)
                    pt2 = psum.tile([128, 16], bf16, tag="tp2a")
                    nc.tensor.transpose(pt2[0:nw, :], aT[:, n0:n0 + nw],
                                        ident_s[0:16, 0:16])
                    nc.vector.tensor_copy(stg1[0:nw, AS1_OFF:AS1_OFF + 16],
                                          pt2[0:nw, :])
                    nc.vector.memset(stg1[:, AS1_OFF + 16:TROW], 0.0)
                    nc.sync.dma_start(
                        tbl_shard[:].rearrange("(w p) t -> p w t", p=128)[:, w, :],
                        stg1[:])

            nc.gpsimd.collective_compute(
                "AllGather", ALU.bypass,
                replica_groups=[list(range(NCORES))],
                ins=[tbl_shard[:].opt()], outs=[tbl_full[:].opt()])
            nc.sync.dma_start(
                adw[:].rearrange("p (w h) -> p w h", w=W),
                tbl_shard[:].rearrange("(w p) t -> p w t", p=128)
                [:, :, AD1_OFF:AD1_OFF + 8])

            with tc.tile_pool(name="f1", bufs=2) as fpool, \
                 tc.tile_pool(name="f1p", bufs=1, space="PSUM") as fpsum:
                def fin1(ch, w0, wn, accc):
                    accv = accc[:, 0:wn * 72].rearrange(
                        "p (w h x) -> p w h x", w=wn, h=8)
                    den = fpool.tile([128, CHW * 8], f32, tag="den")
                    dv = den[:, 0:wn * 8].rearrange("p (w h) -> p w h", w=wn)
                    nc.vector.tensor_scalar_add(dv, accv[:, :, :, 8], EPS)
                    nc.vector.reciprocal(den[:, 0:wn * 8], den[:, 0:wn * 8])
                    hf = fpool.tile([128, CHW * 64], f32, tag="hf")
                    hfv = hf[:, 0:wn * 64].rearrange(
                        "p (w h x) -> p w h x", w=wn, h=8)
                    nc.vector.tensor_tensor(
                        out=hfv, in0=accv[:, :, :, 0:8],
                        in1=den[:, 0:wn * 8]
                            .rearrange("p (w h x) -> p w h x", w=wn, x=1)
                            .to_broadcast([128, wn, 8, 8]),
                        op=ALU.mult)
                    nc.vector.tensor_tensor(
                        out=hf[:, 0:wn * 64].rearrange("p (w x) -> p w x", w=wn),
                        in0=hf[:, 0:wn * 64].rearrange("p (w x) -> p w x", w=wn),
                        in1=b1_s[:].rearrange("p (o x) -> p o x", o=1)
                            .to_broadcast([128, wn, 64]),
                        op=ALU.add)
                    t2 = fpool.tile([128, CHW * 64], f32, tag="t2")
                    nc.vector.tensor_scalar_min(t2[:, 0:wn * 64],
                                                hf[:, 0:wn * 64], 0.0)
                    nc.scalar.activation(t2[:, 0:wn * 64], t2[:, 0:wn * 64],
                                         AF.Exp)
                    nc.vector.tensor_scalar_add(t2[:, 0:wn * 64],
                                                t2[:, 0:wn * 64], -1.0)
                    nc.vector.tensor_scalar_min(t2[:, 0:wn * 64],
                                                t2[:, 0:wn * 64], 0.0)
                    nc.vector.tensor_scalar_max(hf[:, 0:wn * 64],
                                                hf[:, 0:wn * 64], 0.0)
                    h2c = fpool.tile([128, CHW * 64], bf16, tag="h2c")
                    nc.vector.tensor_tensor(out=h2c[:, 0:wn * 64],
                                            in0=hf[:, 0:wn * 64],
                                            in1=t2[:, 0:wn * 64], op=ALU.add)
                    # transpose h2 chunk -> [64, wn*128]
                    ptt = fpsum.tile([128, 512], bf16, tag="ft")
                    h2tc = fpool.tile([64, CHW * 128], bf16, tag="h2tc")
                    for i0 in range(0, wn, 4):
                        infn = min(4, wn - i0)
                        for i in range(infn):
                            nc.tensor.transpose(
                                ptt[0:64, (i0 + i) % 4 * 128:
                                    ((i0 + i) % 4 + 1) * 128],
                                h2c[:, (i0 + i) * 64:(i0 + i + 1) * 64],
                                ident_s[:])
                        nc.scalar.activation(
                            h2tc[:, i0 * 128:(i0 + infn) * 128],
                            ptt[0:64, 0:infn * 128], AF.Copy)
                    # gT / a2T for the chunk
                    gTc = fpool.tile([NCLS, CHW * 128], bf16, tag="gTc")
                    a2Tc = fpool.tile([2, CHW * 128], bf16, tag="a2Tc")
                    psg = fpsum.tile([42, 512], f32, tag="fg")
                    for s0 in range(0, wn * 128, 512):
                        sn = min(512, wn * 128 - s0)
                        nc.tensor.matmul(psg[0:NCLS, 0:sn], lhsT=w2_s[:],
                                         rhs=h2tc[:, s0:s0 + sn],
                                         start=True, stop=True)
                        nc.scalar.activation(gTc[:, s0:s0 + sn],
                                             psg[0:NCLS, 0:sn], AF.Copy)
                        nc.tensor.matmul(psg[40:42, 0:sn], lhsT=att2_s[:],
                                         rhs=gTc[:, s0:s0 + sn],
                                         start=True, stop=True)
                        nc.vector.tensor_copy(a2Tc[:, s0:s0 + sn],
                                              psg[40:42, 0:sn])
                    # stage rows into tbl_shard
                    for i0 in range(0, wn, 4):
                        infn = min(4, wn - i0)
                        stg = fpool.tile([128, 4, TROW], bf16, tag="stg2")
                        nc.vector.memset(stg[:], 0.0)
                        ptg = fpsum.tile([128, 512], bf16, tag="ft")
                        for i in range(infn):
                            nc.tensor.transpose(
                                ptg[:, i * 128:i * 128 + NCLS],
                                gTc[:, (i0 + i) * 128:(i0 + i + 1) * 128],
                                ident_s[0:NCLS, 0:NCLS])
                            nc.tensor.transpose(
                                ptg[:, i * 128 + 64:i * 128 + 66],
                                a2Tc[:, (i0 + i) * 128:(i0 + i + 1) * 128],
                                ident_s[0:2, 0:2])
                        nc.scalar.activation(
                            stg[:, 0:infn, 0:NCLS],
                            ptg[:, 0:infn * 128]
                            .rearrange("p (w c) -> p w c", w=infn)
                            [:, :, 0:NCLS], AF.Copy)
                        nc.vector.memset(stg[:, :, NCLS:NCLS + 1], 1.0)
                        nc.vector.tensor_copy(
                            stg[:, 0:infn, AS2_OFF:AS2_OFF + 1],
                            ptg[:, 0:infn * 128]
                            .rearrange("p (w c) -> p w c", w=infn)
                            [:, :, 64:65])
                        nc.vector.tensor_copy(
                            stg[:, 0:infn, AD2_OFF:AD2_OFF + 1],
                            ptg[:, 0:infn * 128]
                            .rearrange("p (w c) -> p w c", w=infn)
                            [:, :, 65:66])
                        nc.sync.dma_start(
                            tbl_shard[:].rearrange("(w p) t -> p w t", p=128)
                            [:, w0 + i0:w0 + i0 + infn, :],
                            stg[:, 0:infn, :])

                _edge_phase(nc, tc, meta, tbl_full, idxs, drel_s, rconst_s,
                            ident_s, adw, layer=1, finalize=fin1)

            nc.gpsimd.collective_compute(
                "AllGather", ALU.bypass,
                replica_groups=[list(range(NCORES))],
                ins=[tbl_shard[:].opt()], outs=[tbl_full[:].opt()])
            nc.sync.dma_start(
                adw[:, 0:W].rearrange("p (w h) -> p w h", w=W),
                tbl_shard[:].rearrange("(w p) t -> p w t", p=128)
                [:, :, AD2_OFF:AD2_OFF + 1])

            acc2 = pp.tile([128, W * 41], f32, tag="acc")
            _edge_phase(nc, tc, meta, tbl_full, idxs, drel_s, rconst_s,
                        ident_s, adw, acc2, layer=2)

            with tc.tile_pool(name="f2", bufs=1) as pool:
                accv = acc2[:].rearrange("p (w x) -> p w x", w=W)
                den = pool.tile([128, W], f32)
                nc.vector.tensor_scalar_add(den[:], accv[:, :, 40], EPS)
                nc.vector.reciprocal(den[:], den[:])
                o = pool.tile([128, W * NCLS], f32)
                ov = o[:].rearrange("p (w x) -> p w x", w=W)
                nc.vector.tensor_tensor(
                    out=ov, in0=accv[:, :, 0:NCLS],
                    in1=den[:].rearrange("p (w x) -> p w x", x=1)
                        .to_broadcast([128, W, NCLS]),
                    op=ALU.mult)
                nc.vector.tensor_tensor(
                    out=ov, in0=ov,
                    in1=b2_s[:].rearrange("p (o x) -> p o x", o=1)
                        .to_broadcast([128, W, NCLS]),
                    op=ALU.add)
                mx = pool.tile([128, W], f32)
                nc.vector.tensor_reduce(out=mx[:], in_=ov, op=ALU.max,
                                        axis=mybir.AxisListType.X)
                nc.vector.tensor_tensor(
                    out=ov, in0=ov,
                    in1=mx[:].rearrange("p (w x) -> p w x", x=1)
                        .to_broadcast([128, W, NCLS]),
                    op=ALU.subtract)
                nc.scalar.activation(o[:], o[:], AF.Exp)
                sm = pool.tile([128, W], f32)
                nc.vector.tensor_reduce(out=sm[:], in_=ov, op=ALU.add,
                                        axis=mybir.AxisListType.X)
                nc.vector.reciprocal(sm[:], sm[:])
                nc.vector.tensor_tensor(
                    out=ov, in0=ov,
                    in1=sm[:].rearrange("p (w x) -> p w x", x=1)
                        .to_broadcast([128, W, NCLS]),
                    op=ALU.mult)
                nc.sync.dma_start(
                    out[:].rearrange("(w p) x -> p w x", p=128), ov)
    nc.finalize()
    return nc


# ---------------------------------------------------------------- entry point
def kernel(**inputs):
    edge = np.asarray(inputs["edge_index"])
    key = hash(edge[:, :1024].tobytes()) ^ hash(edge.shape)
    if key not in _CACHE:
        meta = _prep(edge)
        nc = _build(meta)
        _CACHE[key] = (meta, nc)
    meta, nc = _CACHE[key]
    maps = _build_inputs(meta, inputs)
    res = bass_utils.run_bass_kernel_spmd(
        nc, maps, core_ids=list(range(NCORES)), trace=False)
    out = np.zeros((N, NCLS), np.float32)
    for core in range(NCORES):
        o = np.asarray(res.results[core]["out"]).reshape(SHARD_PAD, NCLS)
        out[core * SHARD:(core + 1) * SHARD] = o[:SHARD]
    return out


# revision 31
# speedup vs baseline: 1.1554x; 1.1554x over previous
"""2-layer GAT on 8 Trainium2 NeuronCores.

Strategy: dst-shard nodes across cores; per-edge node-feature access via
dma_gather from a bf16 node table (built on device, AllGathered); segment
softmax + aggregation via one-hot selection matmuls on TensorE.

v1: gathers grouped by (chunk-of-windows, quarter) to amortize the large
fixed per-call dma_gather cost; double-buffered chunk pipeline; leaky-relu
via exp(lrelu(x)) = max(exp(x), exp(0.2x)) on ScalarE.
"""
import numpy as np
import ml_dtypes

import concourse.bacc as bacc
import concourse.bass as bass
import concourse.mybir as mybir
import concourse.tile as tile
from concourse import bass_utils

BF = ml_dtypes.bfloat16
bf16 = mybir.dt.bfloat16
f32 = mybir.dt.float32
i16 = mybir.dt.int16

N = 100000
NCORES = 8
SHARD = N // NCORES           # 12500
WIN = 128
W = (SHARD + WIN - 1) // WIN  # 98
SHARD_PAD = W * WIN           # 12544
NQ = 4
QROWS = 2 * SHARD_PAD         # 25088 rows per gather quarter (< 32768)
TROW = 128                    # bf16 elems per table row (256B)
FIN = 512
NCLS = 40
CHW = 7                       # windows per gather chunk
NCH = (W + CHW - 1) // CHW    # 14
AS1_OFF, AD1_OFF = 72, 80
AS2_OFF, AD2_OFF = 48, 56
EPS = 1e-16
AF = mybir.ActivationFunctionType
ALU = mybir.AluOpType

_CACHE = {}


# ---------------------------------------------------------------- host prep
def _prep(edge_index):
    src = np.concatenate([np.asarray(edge_index[0], np.int64),
                          np.arange(N, dtype=np.int64)])
    dst = np.concatenate([np.asarray(edge_index[1], np.int64),
                          np.arange(N, dtype=np.int64)])
    row = (src // SHARD) * SHARD_PAD + (src % SHARD)
    quarter = row // QROWS
    core = dst // SHARD
    dstloc = dst % SHARD
    win = dstloc // WIN
    dstrel = dstloc % WIN
    chunk = win // CHW

    order = np.lexsort((dstrel, win, quarter, core))
    row_s, rel_s = row[order], dstrel[order]

    # cell = (quarter, win); static size = x128 of max count over cores
    cell_id = (core * NQ + quarter) * W + win
    counts = np.bincount(cell_id, minlength=NCORES * NQ * W) \
        .reshape(NCORES, NQ, W)
    kq = (counts.max(axis=0) + 127) // 128              # [NQ, W] blocks

    # chunk layout: for each chunk: quarters in order, cells of its windows
    cell_blk = np.zeros((NQ, W), np.int64)    # chunk-local block offset
    cq_blk = np.zeros((NCH, NQ), np.int64)    # chunk-local block offset of q
    ch_blocks = np.zeros(NCH, np.int64)
    for ch in range(NCH):
        w0, w1 = ch * CHW, min((ch + 1) * CHW, W)
        off = 0
        for q in range(NQ):
            cq_blk[ch, q] = off
            for w in range(w0, w1):
                cell_blk[q, w] = off
                off += kq[q, w]
        ch_blocks[ch] = off
    ch_blk_off = np.zeros(NCH, np.int64)      # global block offset of chunk
    ch_blk_off[1:] = np.cumsum(ch_blocks)[:-1]
    total_blocks = int(ch_blocks.sum())
    S = total_blocks * 128

    idx16 = np.zeros((NCORES, S), np.int16)
    relv = np.full((NCORES, S), -1.0, np.float32)
    starts = np.zeros(NCORES * NQ * W + 1, np.int64)
    np.cumsum(np.bincount(cell_id, minlength=NCORES * NQ * W), out=starts[1:])
    for c in range(NCORES):
        for q in range(NQ):
            for w in range(W):
                ch = w // CHW
                cid = (c * NQ + q) * W + w
                s0, s1 = starts[cid], starts[cid + 1]
                n = s1 - s0
                o = (ch_blk_off[ch] + cell_blk[q, w]) * 128
                idx16[c, o:o + n] = (row_s[s0:s1] - q * QROWS).astype(np.int16)
                relv[c, o:o + n] = rel_s[s0:s1].astype(np.float32)

    # per-window runs: (q, chunk-local blk0, kq) with kq>0; K = total blocks
    win_runs = []
    K = np.zeros(W, np.int64)
    for w in range(W):
        runs = []
        for q in range(NQ):
            if kq[q, w] > 0:
                runs.append((q, int(cell_blk[q, w]), int(kq[q, w])))
        win_runs.append(runs)
        K[w] = kq[:, w].sum()

    return {
        "idx16": idx16, "dstrel": relv, "win_runs": win_runs, "K": K,
        "ch_blocks": ch_blocks, "ch_blk_off": ch_blk_off, "cq_blk": cq_blk,
        "total_blocks": total_blocks, "S": S,
    }


def _build_inputs(meta, inputs):
    x = np.asarray(inputs["x"], np.float32)
    W1 = np.asarray(inputs["W1"], np.float32)
    W2 = np.asarray(inputs["W2"], np.float32)
    as1 = np.asarray(inputs["att_src1"], np.float32).reshape(8, 8)
    ad1 = np.asarray(inputs["att_dst1"], np.float32).reshape(8, 8)
    as2 = np.asarray(inputs["att_src2"], np.float32).reshape(NCLS)
    ad2 = np.asarray(inputs["att_dst2"], np.float32).reshape(NCLS)
    b1 = np.asarray(inputs["b1"], np.float32)
    b2 = np.asarray(inputs["b2"], np.float32)

    attsd = np.zeros((64, 16), np.float32)
    for h in range(8):
        attsd[h * 8:(h + 1) * 8, h] = as1[h]
        attsd[h * 8:(h + 1) * 8, 8 + h] = ad1[h]
    att2sd = np.stack([as2, ad2], axis=1)

    common = {
        "w1": W1.astype(BF),
        "w2": W2.astype(BF),
        "attsd": attsd.astype(BF),
        "att2sd": att2sd.astype(BF),
        "b1c": np.tile(b1[None, :], (128, 1)).astype(np.float32),
        "b2c": np.tile(b2[None, :], (128, 1)).astype(np.float32),
        "rconst": np.tile(np.arange(128, dtype=np.float32)[None, :],
                          (128, 1)).astype(BF),
        "ident": np.eye(128, dtype=np.float32).astype(BF),
    }
    S = int(meta["S"])
    maps = []
    for core in range(NCORES):
        idx = meta["idx16"][core]
        idx_in = np.tile(idx.reshape(S // 16, 16).T, (8, 1))
        drel_in = meta["dstrel"][core].reshape(S // 128, 128).T.astype(BF)
        m = dict(common)
        m["xT"] = np.ascontiguousarray(
            x[core * SHARD:(core + 1) * SHARD].T).astype(BF)
        m["idxs"] = np.ascontiguousarray(idx_in)
        m["drel"] = np.ascontiguousarray(drel_in)
        maps.append(m)
    return maps


# ---------------------------------------------------------------- bass build
def _edge_phase(nc, tc, meta, tbl_full, idxs, drel_s, rconst_s, ident_s,
                adw, layer, finalize):
    K = meta["K"]
    ch_blocks, ch_blk_off = meta["ch_blocks"], meta["ch_blk_off"]
    cq_blk, win_runs = meta["cq_blk"], meta["win_runs"]
    H = 8 if layer == 1 else 1
    VPW = 72 if layer == 1 else 41
    GW = 9 if layer == 1 else 41
    AOFF = AS1_OFF if layer == 1 else AS2_OFF
    CBMAX = int(ch_blocks.max())
    KWMAX = int(K.max())
    with tc.tile_pool(name=f"ep{layer}", bufs=4) as pool, \
         tc.tile_pool(name=f"eg{layer}", bufs=2) as gpool, \
         tc.tile_pool(name=f"ea{layer}", bufs=2) as apool, \
         tc.tile_pool(name=f"epa{layer}", bufs=2, space="PSUM") as psa, \
         tc.tile_pool(name=f"epq{layer}", bufs=2, space="PSUM") as psq, \
         tc.tile_pool(name=f"epd{layer}", bufs=2, space="PSUM") as psd:
        for ch in range(NCH):
            chb = int(ch_blocks[ch])
            gblk0 = int(ch_blk_off[ch])
            idx_t = gpool.tile([128, CBMAX * 8], i16, tag="idx")
            nc.sync.dma_start(idx_t[:, 0:chb * 8],
                              idxs[:, gblk0 * 8:(gblk0 + chb) * 8])
            g = gpool.tile([128, CBMAX, TROW], bf16, tag="gather")
            for q in range(NQ):
                b0 = int(cq_blk[ch, q])
                b1 = int(cq_blk[ch, q + 1]) if q + 1 < NQ else chb
                nbq = b1 - b0
                if nbq == 0:
                    continue
                nc.gpsimd.dma_gather(
                    out_ap=g[:, b0:b1, :],
                    in_ap=tbl_full[QROWS * q:min(QROWS * (q + 1),
                                                 NCORES * SHARD_PAD), :],
                    idxs_ap=idx_t[:, b0 * 8:b1 * 8],
                    num_idxs=nbq * 128,
                    num_idxs_reg=nbq * 128,
                    elem_size=TROW,
                    single_packet=False,
                    queue_num=q)
            w0ch = ch * CHW
            wnch = min(CHW, W - w0ch)
            accc = apool.tile([128, CHW * VPW], f32, tag="acc")
            for w in range(w0ch, w0ch + wnch):
                kw = int(K[w])
                runs = win_runs[w]
                adww = adw[:].rearrange("p (w h) -> p w h", w=W)[:, w, 0:H] \
                    if layer == 1 else adw[:, w:w + 1]
                sel = pool.tile([128, KWMAX * 128], bf16, tag="sel")
                sp = 0
                for (q, lb0, kqw) in runs:
                    nc.vector.tensor_tensor(
                        out=sel[:, sp * 128:(sp + kqw) * 128]
                            .rearrange("p (k j) -> p k j", k=kqw),
                        in0=drel_s[:, gblk0 + lb0:gblk0 + lb0 + kqw]
                            .rearrange("p (k x) -> p k x", x=1)
                            .to_broadcast([128, kqw, 128]),
                        in1=rconst_s[:].rearrange("p (x j) -> p x j", x=1)
                            .to_broadcast([128, kqw, 128]),
                        op=ALU.is_equal)
                    sp += kqw
                ep = pool.tile([128, KWMAX * H], f32, tag="ep")
                sp = 0
                for (q, lb0, kqw) in runs:
                    for k0 in range(0, kqw, 8):
                        kn = min(8, kqw - k0)
                        pst = psq.tile([128, 8 * 128], bf16, tag="selT")
                        for k in range(kn):
                            nc.tensor.transpose(
                                pst[:, k * 128:(k + 1) * 128],
                                sel[:, (sp + k0 + k) * 128:
                                    (sp + k0 + k + 1) * 128], ident_s[:])
                        selt = pool.tile([128, 8 * 128], bf16, tag="selt")
                        nc.scalar.activation(selt[:, 0:kn * 128],
                                             pst[:, 0:kn * 128], AF.Copy)
                        pad_ = psd.tile([128, 8 * H], f32, tag="adst")
                        for k in range(kn):
                            nc.tensor.matmul(
                                pad_[:, k * H:(k + 1) * H],
                                lhsT=selt[:, k * 128:(k + 1) * 128],
                                rhs=adww, start=True, stop=True)
                        nc.vector.tensor_tensor(
                            out=ep[:, (sp + k0) * H:(sp + k0 + kn) * H]
                                .rearrange("p (k h) -> p k h", k=kn),
                            in0=g[:, lb0 + k0:lb0 + k0 + kn, AOFF:AOFF + H],
                            in1=pad_[:, 0:kn * H]
                                .rearrange("p (k h) -> p k h", k=kn),
                            op=ALU.add)
                    sp += kqw
                # exp(lrelu(x)) = max(exp(x), exp(0.2x))
                pex = pool.tile([128, KWMAX * H], bf16, tag="pex")
                pex2 = pool.tile([128, KWMAX * H], bf16, tag="pex2")
                nc.scalar.activation(pex[:, 0:kw * H], ep[:, 0:kw * H], AF.Exp)
                nc.scalar.activation(pex2[:, 0:kw * H], ep[:, 0:kw * H],
                                     AF.Exp, scale=0.2)
                nc.vector.tensor_tensor(out=pex[:, 0:kw * H],
                                        in0=pex[:, 0:kw * H],
                                        in1=pex2[:, 0:kw * H], op=ALU.max)
                vp = pool.tile([128, KWMAX, VPW], bf16, tag="vp")
                sp = 0
                for (q, lb0, kqw) in runs:
                    nc.vector.tensor_tensor(
                        out=vp[:, sp:sp + kqw, :]
                            .rearrange("p k (h x) -> p k h x", h=H),
                        in0=g[:, lb0:lb0 + kqw, 0:VPW]
                            .rearrange("p k (h x) -> p k h x", h=H),
                        in1=pex[:, sp * H:(sp + kqw) * H]
                            .rearrange("p (k h x) -> p k h x", k=kqw, x=1)
                            .to_broadcast([128, kqw, H, GW]),
                        op=ALU.mult)
                    sp += kqw
                pagg = psa.tile([128, VPW], f32, tag="agg")
                for k in range(kw):
                    nc.tensor.matmul(pagg[:], lhsT=sel[:, k * 128:(k + 1) * 128],
                                     rhs=vp[:, k, :], start=(k == 0),
                                     stop=(k == kw - 1))
                nc.vector.tensor_copy(
                    accc[:, (w - w0ch) * VPW:(w - w0ch + 1) * VPW], pagg[:])
            finalize(ch, w0ch, wnch, accc)


def _build(meta):
    S = int(meta["S"])
    KT = FIN // 128

    nc = bacc.Bacc("TRN2", target_bir_lowering=False, debug=False,
                   num_devices=NCORES, num_swdge_queues=4)
    xT = nc.dram_tensor("xT", [FIN, SHARD], bf16, kind="ExternalInput")
    w1 = nc.dram_tensor("w1", [FIN, 64], bf16, kind="ExternalInput")
    w2 = nc.dram_tensor("w2", [64, NCLS], bf16, kind="ExternalInput")
    attsd = nc.dram_tensor("attsd", [64, 16], bf16, kind="ExternalInput")
    att2sd = nc.dram_tensor("att2sd", [NCLS, 2], bf16, kind="ExternalInput")
    b1c = nc.dram_tensor("b1c", [128, 64], f32, kind="ExternalInput")
    b2c = nc.dram_tensor("b2c", [128, NCLS], f32, kind="ExternalInput")
    rconst = nc.dram_tensor("rconst", [128, 128], bf16, kind="ExternalInput")
    ident = nc.dram_tensor("ident", [128, 128], bf16, kind="ExternalInput")
    idxs = nc.dram_tensor("idxs", [128, S // 16], i16, kind="ExternalInput")
    drel = nc.dram_tensor("drel", [128, S // 128], bf16, kind="ExternalInput")
    out = nc.dram_tensor("out", [SHARD_PAD, NCLS], f32, kind="ExternalOutput")

    with tile.TileContext(nc) as tc:
        with tc.tile_pool(name="dram", bufs=1, space="DRAM") as dpool, \
             tc.tile_pool(name="persist", bufs=1) as pp:
            tbl_shard = dpool.tile([SHARD_PAD, TROW], bf16)
            tbl_full = dpool.tile([NCORES * SHARD_PAD, TROW], bf16,
                                  addr_space="Shared")
            tbl_full2 = dpool.tile([NCORES * SHARD_PAD, TROW], bf16,
                                   addr_space="Shared")

            ident_s = pp.tile([128, 128], bf16)
            nc.sync.dma_start(ident_s[:], ident[:])
            rconst_s = pp.tile([128, 128], bf16)
            nc.sync.dma_start(rconst_s[:], rconst[:])
            drel_s = pp.tile([128, S // 128], bf16)
            nc.sync.dma_start(drel_s[:], drel[:])
            b1_s = pp.tile([128, 64], f32)
            nc.sync.dma_start(b1_s[:], b1c[:])
            b2_s = pp.tile([128, NCLS], f32)
            nc.sync.dma_start(b2_s[:], b2c[:])
            w2_s = pp.tile([64, NCLS], bf16)
            nc.sync.dma_start(w2_s[:], w2[:])
            att2_s = pp.tile([NCLS, 2], bf16)
            nc.sync.dma_start(att2_s[:], att2sd[:])
            adw = pp.tile([128, W * 8], bf16, tag="adw")

            # phase A: hT = W1^T x^T, aT = att^T hT; build node table layer 1
            with tc.tile_pool(name="pa", bufs=2) as pool, \
                 tc.tile_pool(name="pac", bufs=1) as cpool, \
                 tc.tile_pool(name="pap", bufs=1, space="PSUM") as psum:
                w1_s = cpool.tile([128, KT, 64], bf16)
                nc.sync.dma_start(w1_s[:], w1[:].rearrange("(k p) m -> p k m", p=128))
                att_s = cpool.tile([64, 16], bf16)
                nc.sync.dma_start(att_s[:], attsd[:])
                hT = cpool.tile([64, SHARD], bf16)
                aT = cpool.tile([16, SHARD], bf16)
                CH = 500
                for ci in range(SHARD // CH):
                    s0, s1 = ci * CH, (ci + 1) * CH
                    xt_t = pool.tile([128, KT, CH], bf16, tag="xt")
                    nc.sync.dma_start(
                        xt_t[:], xT[:, s0:s1].rearrange("(k p) n -> p k n", p=128))
                    ps = psum.tile([64, CH], f32, tag="hps")
                    for k in range(KT):
                        nc.tensor.matmul(ps[:], lhsT=w1_s[:, k, :],
                                         rhs=xt_t[:, k, :],
                                         start=(k == 0), stop=(k == KT - 1))
                    nc.vector.tensor_copy(hT[:, s0:s1], ps[:])
                    ps2 = psum.tile([16, CH], f32, tag="aps")
                    nc.tensor.matmul(ps2[:], lhsT=att_s[:], rhs=hT[:, s0:s1],
                                     start=True, stop=True)
                    nc.vector.tensor_copy(aT[:, s0:s1], ps2[:])
                WFULL = (SHARD // 128) & ~3  # full 4-window groups
                for w0 in range(0, WFULL, 4):
                    stg = pool.tile([128, 4, TROW], bf16, tag="stg")
                    pt = psum.tile([128, 4 * 64], bf16, tag="tp")
                    pt2 = psum.tile([128, 4 * 16], bf16, tag="tp2")
                    for i in range(4):
                        n0 = (w0 + i) * 128
                        nc.tensor.transpose(pt[:, i * 64:i * 64 + 64],
                                            hT[:, n0:n0 + 128],
                                            ident_s[0:64, 0:64])
                        nc.tensor.transpose(pt2[:, i * 16:i * 16 + 16],
                                            aT[:, n0:n0 + 128],
                                            ident_s[0:16, 0:16])
                    nc.vector.tensor_copy(
                        stg[:, :, 0:72].rearrange("p w (h c) -> p w h c", h=8)
                        [:, :, :, 0:8],
                        pt[:].rearrange("p (w h c) -> p w h c", w=4, h=8))
                    nc.vector.memset(
                        stg[:, :, 0:72].rearrange("p w (h c) -> p w h c", h=8)
                        [:, :, :, 8], 1.0)
                    nc.scalar.activation(
                        stg[:, :, AS1_OFF:AS1_OFF + 16],
                        pt2[:].rearrange("p (w c) -> p w c", w=4),
                        AF.Copy)
                    nc.vector.memset(stg[:, :, AS1_OFF + 16:TROW], 0.0)
                    nc.sync.dma_start(
                        tbl_shard[:].rearrange("(w p) t -> p w t", p=128)
                        [:, w0:w0 + 4, :],
                        stg[:])
                for w in range(WFULL, W):
                    stg1 = pool.tile([128, TROW], bf16, tag="stg1")
                    n0 = w * 128
                    nw = min(128, SHARD - n0)
                    if nw < 128:
                        nc.vector.memset(stg1[:], 0.0)
                    pt = psum.tile([128, 64], bf16, tag="tp1")
                    nc.tensor.transpose(pt[0:nw, 0:64], hT[:, n0:n0 + nw],
                                        ident_s[0:64, 0:64])
                    nc.vector.tensor_copy(
                        stg1[0:nw, 0:72].rearrange("p (h c) -> p h c", h=8)[:, :, 0:8],
                        pt[0:nw, 0:64].rearrange("p (h c) -> p h c", h=8))
                    nc.vector.memset(
                        stg1[:, 0:72].rearrange("p (h c) -> p h c", h=8)[:, :, 8], 1.0)
                    pt2 = psum.tile([128, 16], bf16, tag="tp2a")
                    nc.tensor.transpose(pt2[0:nw, :], aT[:, n0:n0 + nw],
                                        ident_s[0:16, 0:16])
                    nc.vector.tensor_copy(stg1[0:nw, AS1_OFF:AS1_OFF + 16],
                                          pt2[0:nw, :])
                    nc.vector.memset(stg1[:, AS1_OFF + 16:TROW], 0.0)
                    nc.sync.dma_start(
                        tbl_shard[:].rearrange("(w p) t -> p w t", p=128)[:, w, :],
                        stg1[:])

            nc.gpsimd.collective_compute(
                "AllGather", ALU.bypass,
                replica_groups=[list(range(NCORES))],
                ins=[tbl_shard[:].opt()], outs=[tbl_full[:].opt()])
            nc.sync.dma_start(
                adw[:].rearrange("p (w h) -> p w h", w=W),
                tbl_shard[:].rearrange("(w p) t -> p w t", p=128)
                [:, :, AD1_OFF:AD1_OFF + 8])

            with tc.tile_pool(name="f1", bufs=2) as fpool, \
                 tc.tile_pool(name="f1p", bufs=1, space="PSUM") as fpsum:
                def fin1(ch, w0, wn, accc):
                    accv = accc[:, 0:wn * 72].rearrange(
                        "p (w h x) -> p w h x", w=wn, h=8)
                    den = fpool.tile([128, CHW * 8], f32, tag="den")
                    dv = den[:, 0:wn * 8].rearrange("p (w h) -> p w h", w=wn)
                    nc.vector.tensor_scalar_add(dv, accv[:, :, :, 8], EPS)
                    nc.vector.reciprocal(den[:, 0:wn * 8], den[:, 0:wn * 8])
                    hf = fpool.tile([128, CHW * 64], f32, tag="hf")
                    hfv = hf[:, 0:wn * 64].rearrange(
                        "p (w h x) -> p w h x", w=wn, h=8)
                    nc.vector.tensor_tensor(
                        out=hfv, in0=accv[:, :, :, 0:8],
                        in1=den[:, 0:wn * 8]
                            .rearrange("p (w h x) -> p w h x", w=wn, x=1)
                            .to_broadcast([128, wn, 8, 8]),
                        op=ALU.mult)
                    nc.vector.tensor_tensor(
                        out=hf[:, 0:wn * 64].rearrange("p (w x) -> p w x", w=wn),
                        in0=hf[:, 0:wn * 64].rearrange("p (w x) -> p w x", w=wn),
                        in1=b1_s[:].rearrange("p (o x) -> p o x", o=1)
                            .to_broadcast([128, wn, 64]),
                        op=ALU.add)
                    t2 = fpool.tile([128, CHW * 64], f32, tag="t2")
                    nc.vector.tensor_scalar_min(t2[:, 0:wn * 64],
                                                hf[:, 0:wn * 64], 0.0)
                    nc.scalar.activation(t2[:, 0:wn * 64], t2[:, 0:wn * 64],
                                         AF.Exp)
                    nc.vector.tensor_scalar_add(t2[:, 0:wn * 64],
                                                t2[:, 0:wn * 64], -1.0)
                    nc.vector.tensor_scalar_min(t2[:, 0:wn * 64],
                                                t2[:, 0:wn * 64], 0.0)
                    nc.vector.tensor_scalar_max(hf[:, 0:wn * 64],
                                                hf[:, 0:wn * 64], 0.0)
                    h2c = fpool.tile([128, CHW * 64], bf16, tag="h2c")
                    nc.vector.tensor_tensor(out=h2c[:, 0:wn * 64],
                                            in0=hf[:, 0:wn * 64],
                                            in1=t2[:, 0:wn * 64], op=ALU.add)
                    # transpose h2 chunk -> [64, wn*128]
                    ptt = fpsum.tile([128, 512], bf16, tag="ft")
                    h2tc = fpool.tile([64, CHW * 128], bf16, tag="h2tc")
                    for i0 in range(0, wn, 4):
                        infn = min(4, wn - i0)
                        for i in range(infn):
                            nc.tensor.transpose(
                                ptt[0:64, (i0 + i) % 4 * 128:
                                    ((i0 + i) % 4 + 1) * 128],
                                h2c[:, (i0 + i) * 64:(i0 + i + 1) * 64],
                                ident_s[:])
                        nc.scalar.activation(
                            h2tc[:, i0 * 128:(i0 + infn) * 128],
                            ptt[0:64, 0:infn * 128], AF.Copy)
                    # gT / a2T for the chunk
                    gTc = fpool.tile([NCLS, CHW * 128], bf16, tag="gTc")
                    a2Tc = fpool.tile([2, CHW * 128], bf16, tag="a2Tc")
                    psg = fpsum.tile([128, 512], f32, tag="fg")
                    for s0 in range(0, wn * 128, 512):
                        sn = min(512, wn * 128 - s0)
                        nc.tensor.matmul(psg[0:NCLS, 0:sn], lhsT=w2_s[:],
                                         rhs=h2tc[:, s0:s0 + sn],
                                         start=True, stop=True)
                        nc.scalar.activation(gTc[:, s0:s0 + sn],
                                             psg[0:NCLS, 0:sn], AF.Copy)
                        nc.tensor.matmul(psg[64:66, 0:sn], lhsT=att2_s[:],
                                         rhs=gTc[:, s0:s0 + sn],
                                         start=True, stop=True)
                        nc.vector.tensor_copy(a2Tc[:, s0:s0 + sn],
                                              psg[64:66, 0:sn])
                    # stage rows into tbl_shard
                    for i0 in range(0, wn, 4):
                        infn = min(4, wn - i0)
                        stg = fpool.tile([128, 4, TROW], bf16, tag="stg2")
                        nc.vector.memset(stg[:], 0.0)
                        ptg = fpsum.tile([128, 512], bf16, tag="ft")
                        for i in range(infn):
                            nc.tensor.transpose(
                                ptg[:, i * 128:i * 128 + NCLS],
                                gTc[:, (i0 + i) * 128:(i0 + i + 1) * 128],
                                ident_s[0:NCLS, 0:NCLS])
                            nc.tensor.transpose(
                                ptg[:, i * 128 + 64:i * 128 + 66],
                                a2Tc[:, (i0 + i) * 128:(i0 + i + 1) * 128],
                                ident_s[0:2, 0:2])
                        nc.scalar.activation(
                            stg[:, 0:infn, 0:NCLS],
                            ptg[:, 0:infn * 128]
                            .rearrange("p (w c) -> p w c", w=infn)
                            [:, :, 0:NCLS], AF.Copy)
                        nc.vector.memset(stg[:, :, NCLS:NCLS + 1], 1.0)
                        nc.vector.tensor_copy(
                            stg[:, 0:infn, AS2_OFF:AS2_OFF + 1],
                            ptg[:, 0:infn * 128]
                            .rearrange("p (w c) -> p w c", w=infn)
                            [:, :, 64:65])
                        nc.vector.tensor_copy(
                            stg[:, 0:infn, AD2_OFF:AD2_OFF + 1],
                            ptg[:, 0:infn * 128]
                            .rearrange("p (w c) -> p w c", w=infn)
                            [:, :, 65:66])
                        nc.sync.dma_start(
                            tbl_shard[:].rearrange("(w p) t -> p w t", p=128)
                            [:, w0 + i0:w0 + i0 + infn, :],
                            stg[:, 0:infn, :])

                _edge_phase(nc, tc, meta, tbl_full, idxs, drel_s, rconst_s,
                            ident_s, adw, layer=1, finalize=fin1)

            nc.gpsimd.collective_compute(
                "AllGather", ALU.bypass,
                replica_groups=[list(range(NCORES))],
                ins=[tbl_shard[:].opt()], outs=[tbl_full2[:].opt()])
            nc.sync.dma_start(
                adw[:, 0:W].rearrange("p (w h) -> p w h", w=W),
                tbl_shard[:].rearrange("(w p) t -> p w t", p=128)
                [:, :, AD2_OFF:AD2_OFF + 1])

            with tc.tile_pool(name="f2", bufs=2) as f2pool:
                def fin2(ch, w0, wn, accc):
                    accv = accc[:, 0:wn * 41].rearrange(
                        "p (w x) -> p w x", w=wn)
                    den = f2pool.tile([128, CHW], f32, tag="den")
                    nc.vector.tensor_scalar_add(den[:, 0:wn],
                                                accv[:, :, 40], EPS)
                    nc.vector.reciprocal(den[:, 0:wn], den[:, 0:wn])
                    o = f2pool.tile([128, CHW * NCLS], f32, tag="o")
                    ov = o[:, 0:wn * NCLS].rearrange("p (w x) -> p w x", w=wn)
                    nc.vector.tensor_tensor(
                        out=ov, in0=accv[:, :, 0:NCLS],
                        in1=den[:, 0:wn].rearrange("p (w x) -> p w x", x=1)
                            .to_broadcast([128, wn, NCLS]),
                        op=ALU.mult)
                    nc.vector.tensor_tensor(
                        out=ov, in0=ov,
                        in1=b2_s[:].rearrange("p (o x) -> p o x", o=1)
                            .to_broadcast([128, wn, NCLS]),
                        op=ALU.add)
                    mx = f2pool.tile([128, CHW], f32, tag="mx")
                    nc.vector.tensor_reduce(out=mx[:, 0:wn], in_=ov,
                                            op=ALU.max,
                                            axis=mybir.AxisListType.X)
                    nc.vector.tensor_tensor(
                        out=ov, in0=ov,
                        in1=mx[:, 0:wn].rearrange("p (w x) -> p w x", x=1)
                            .to_broadcast([128, wn, NCLS]),
                        op=ALU.subtract)
                    nc.scalar.activation(o[:, 0:wn * NCLS],
                                         o[:, 0:wn * NCLS], AF.Exp)
                    sm = f2pool.tile([128, CHW], f32, tag="sm")
                    nc.vector.tensor_reduce(out=sm[:, 0:wn], in_=ov,
                                            op=ALU.add,
                                            axis=mybir.AxisListType.X)
                    nc.vector.reciprocal(sm[:, 0:wn], sm[:, 0:wn])
                    nc.vector.tensor_tensor(
                        out=ov, in0=ov,
                        in1=sm[:, 0:wn].rearrange("p (w x) -> p w x", x=1)
                            .to_broadcast([128, wn, NCLS]),
                        op=ALU.mult)
                    nc.sync.dma_start(
                        out[:].rearrange("(w p) x -> p w x", p=128)
                        [:, w0:w0 + wn, :], ov)

                _edge_phase(nc, tc, meta, tbl_full2, idxs, drel_s, rconst_s,
                            ident_s, adw, layer=2, finalize=fin2)
    nc.finalize()
    return nc


# ---------------------------------------------------------------- entry point
def kernel(**inputs):
    edge = np.asarray(inputs["edge_index"])
    key = hash(edge[:, :1024].tobytes()) ^ hash(edge.shape)
    if key not in _CACHE:
        meta = _prep(edge)
        nc = _build(meta)
        _CACHE[key] = (meta, nc)
    meta, nc = _CACHE[key]
    maps = _build_inputs(meta, inputs)
    res = bass_utils.run_bass_kernel_spmd(
        nc, maps, core_ids=list(range(NCORES)), trace=False)
    out = np.zeros((N, NCLS), np.float32)
    for core in range(NCORES):
        o = np.asarray(res.results[core]["out"]).reshape(SHARD_PAD, NCLS)
        out[core * SHARD:(core + 1) * SHARD] = o[:SHARD]
    return out
